# revision 35
# baseline (speedup 1.0000x reference)
"""Causal single-head attention on 8 trn2 NeuronCores - split-72 geometry.

B=4, S=2048, D_MODEL=1024, D_HEAD=64, fp32 in/out.

Sharding: 2 cores per batch with an interleaved query-tile split
(h=0 owns tiles {0,2,4,6,9,11,13,15}, h=1 the complement; 68 causal
128x128 blocks each). The host feeds each core E^T [dm, s] bf16 with
columns ordered [own tiles DESCENDING | other tiles ascending]. With
own-descending query columns, the queries needing key tile at position
p form a PREFIX of the 1024 QT columns, so each score unit computes a
prefix range:
  position p 0..7  (own keys):   width (p+1)*128, diag tri at last block
  position p 8..15 (other keys): width (16-p)*128, last block either
    fully causal or fully dead - killed by a per-core 0/-30000 exp bias
Total 72 blocks/core vs 84 for the contiguous-half split (68 = ideal).

Per-core pipeline (identical SPMD program, all matmuls bf16):
  Warmup N=512 matmuls open the PE HAM clock gate while the first input
  DMA is in flight; inputs stream over BOTH hardware DGE rings (Sync +
  Act). Projections per 512-col chunk of E^T: one [Wv|Wk]-packed pass
  (V^T on PSUM rows 0:64, K^T on 64:128) plus, for the core's own 2
  chunks, a Wq/8 pass targeting PSUM rows 64:128. Q^T/K^T live on SBUF
  partitions 64:128 (shared-base-partition rule); V tiles are
  PE-transposed into Vp [128k, 16, 65] with a ones column (softmax
  denominator). Zero biases -> all projection PSUM->SBUF copies are
  bias-free.
  PV accumulates out^T [65, 1024] in one 2-bank PSUM tile; start=True
  resets a whole 512-col psum bank, so the widest unit of each bank
  (3, 7) is emitted first and opens its bank with one full-bank start,
  everything else accumulates. Output drains in 3 pieces as column
  regions complete; the host does the final divide+transpose+scatter.
"""

import sys

if "/opt/trn_rl_repo" not in sys.path:
    sys.path.insert(0, "/opt/trn_rl_repo")

import numpy as np

B, S, D, H = 4, 2048, 1024, 64
P = 128
KO = D // P          # 8 dmodel chunks
NT = S // P          # 16 seq tiles
NEG = -30000.0
OWN0 = [0, 2, 4, 6, 9, 11, 13, 15]   # h=0 query tiles
OWN1 = [t for t in range(16) if t not in OWN0]


def _order(h):
    own = OWN0 if h == 0 else OWN1
    other = OWN1 if h == 0 else OWN0
    return sorted(own, reverse=True) + sorted(other)


def _width(p):
    return p + 1 if p < 8 else 16 - p


def _build_program(zb):
    import concourse.bacc as bacc
    import concourse.mybir as mybir
    import concourse.tile as tile

    f32 = mybir.dt.float32
    bf16 = mybir.dt.bfloat16
    AF = mybir.ActivationFunctionType
    ALU = mybir.AluOpType

    nc = bacc.Bacc()
    et = nc.declare_dram_parameter("et", [4, P, KO * 512], bf16, isOutput=False)
    # per partition cols 0:1536 = [Wv|Wk|Wq/8] x 8 ko, cols 1536:5632 = chunk0
    wc0 = nc.declare_dram_parameter("wc0", [P, 1536 + 4096], bf16, isOutput=False)
    # cols: bq/8 | bk | bv | bg[8..15] (0 or NEG per core)
    biasg = nc.declare_dram_parameter("biasg", [P, 11], f32, isOutput=False)
    # cols 0:128 = shared tri diag mask, cols 128:192 = identity (rows 0:64)
    mi = nc.declare_dram_parameter("mi", [P, P + H], bf16, isOutput=False)
    out = nc.declare_dram_parameter("out", [H + 1, 1024], f32, isOutput=True)

    from contextlib import ExitStack

    with tile.TileContext(nc) as tc, ExitStack() as ctx:
        cpool = ctx.enter_context(tc.tile_pool(name="const", bufs=1))
        vtp = ctx.enter_context(tc.tile_pool(name="vt", bufs=2))
        ptp = ctx.enter_context(tc.tile_pool(name="pt", bufs=10))
        psb = ctx.enter_context(tc.tile_pool(name="psb", bufs=2, space="PSUM"))

        # --- input DMAs: ONE hardware ring, strict need-order (both DGE
        # rings share HBM bandwidth; splitting starves the critical pieces)
        # weights and chunk0 in small pieces so the first projection ko's
        # unblock as soon as their slice lands (dma deps are per-dma_start)
        wc_sb = cpool.tile([P, 1536 + 4096], bf16, tag="wc0")
        nc.sync.dma_start(wc_sb[:, 0:768], wc0[:, 0:768])
        nc.sync.dma_start(wc_sb[:, 768:1536], wc0[:, 768:1536])
        for a in range(1536, 5632, 1024):
            b = min(a + 1024, 5632)
            nc.sync.dma_start(wc_sb[:, a:b], wc0[:, a:b])
        mi_sb = cpool.tile([P, P + H], bf16, tag="mi")
        nc.sync.dma_start(mi_sb[:], mi[:])
        bias_sb = cpool.tile([P, 11], f32, tag="biasg")
        nc.sync.dma_start(bias_sb[:], biasg[:])
        # ET chunks in ko-halves (contiguous 4KB/partition descriptors):
        # each chunk's first matmuls start ~1.4us before the full chunk
        # lands - kills the straggler utilization dip that fragments the
        # HAM full-clock grant on DMA-jittered cores
        ET = cpool.tile([P, 4, KO, 512], bf16, tag="ET")
        for cc in (1, 2, 3):
            nc.sync.dma_start(ET[:, cc, 0:4, :], et[cc, :, 0:2048])
            nc.sync.dma_start(ET[:, cc, 4:8, :], et[cc, :, 2048:4096])

        def w_ap(ko, a, b):
            return wc_sb[:, ko * 192 + a:ko * 192 + b]

        def et_ap(cc, ko):
            if cc == 0:
                return wc_sb[:, 1536 + ko * 512:1536 + (ko + 1) * 512]
            return ET[:, cc, ko, :]

        bq_sb = bias_sb[:, 0:1]
        bk_sb = bias_sb[:, 1:2]
        bv_sb = bias_sb[:H, 2:3]

        def bg_sb(p):
            return bias_sb[:, 3 + (p - 8):4 + (p - 8)]

        tri_sb = mi_sb[:, 0:P]
        id_sb = mi_sb[:H, P:P + H]

        QT = cpool.tile([P, 1024], bf16, tag="QT")
        KT = cpool.tile([P, S], bf16, tag="KT")
        Vp = cpool.tile([P, NT, H + 1], bf16, tag="Vp")
        o_sb = cpool.tile([H + 1, 1024], f32, tag="osb")
        wtile = cpool.tile([P, 512], bf16, tag="warm")
        nc.vector.memset(wtile[:], 0.0)
        nc.vector.memset(Vp[:, :, H:H + 1], 1.0)

        def vtranspose(vt, cc):
            for t in range(4):
                kt = cc * 4 + t
                pvt = psb.tile([P, H], bf16, tag="pj", name=f"pvt_{kt}")
                nc.tensor.transpose(
                    pvt[:], vt[:, t * P:(t + 1) * P], id_sb[:]
                )
                nc.vector.tensor_copy(Vp[:, kt, :H], pvt[:])

        vts = [None] * 4

        def pcopy(dst, src_ap, bias, eng):
            if zb:
                if eng == "act":
                    nc.scalar.activation(dst, src_ap, AF.Copy)
                else:
                    nc.vector.tensor_copy(dst, src_ap)
            else:
                nc.vector.tensor_scalar_add(dst, src_ap, bias)

        def vk_chunk(cc, halves=False):
            # halves=True: two independent 256-col accumulation groups (in
            # SEPARATE psum tiles - start=True resets a whole bank) so the
            # first half's PSUM->SBUF copies overlap the second half's
            # matmuls - removes the chunk-transition PE bubble
            # KT/vt copies on DVE so they run concurrently with q_chunk's
            # ACT copy - the first scores then wait max(DVE, ACT) not sum
            eng = "dve"
            vt = vtp.tile([H, 512], bf16, tag="vt", name=f"vt_{cc}")
            grps = [(0, 256), (256, 512)] if halves else [(0, 512)]
            for a, b in grps:
                ps = psb.tile([P, b - a], f32, tag="pj",
                              name=f"vk_ps_{cc}_{a}")
                for ko in range(KO):
                    nc.tensor.matmul(
                        ps[:], w_ap(ko, 0, 128), et_ap(cc, ko)[:, a:b],
                        start=(ko == 0), stop=(ko == KO - 1),
                        skip_group_check=True,
                    )
                # KT copy in 128-col pieces: the next chunk's first score
                # unit needs only the first 128 cols, so it unblocks early
                for c0 in range(a, b, P):
                    pcopy(
                        KT[H:P, cc * 512 + c0:cc * 512 + c0 + P],
                        ps[H:P, c0 - a:c0 - a + P], bk_sb[H:P], eng,
                    )
                pcopy(vt[:, a:b], ps[:H, :], bv_sb[:], eng)
            vts[cc] = vt

        def q_chunk(cc):
            ps = psb.tile([P, 512], f32, tag="pj", name=f"q_ps_{cc}")
            for ko in range(KO):
                nc.tensor.matmul(
                    ps[H:P, :], w_ap(ko, 128, 192), et_ap(cc, ko),
                    start=(ko == 0), stop=(ko == KO - 1),
                )
            pcopy(
                QT[H:P, cc * 512:(cc + 1) * 512], ps[H:P, :], bq_sb[H:P],
                "act" if cc == 0 else "dve",
            )

        # --- attention: 16 prefix-range units over one 2-bank out^T psum
        outT = psb.tile([P, 1024], f32, tag="os", bufs=1)
        pts = [None] * NT

        def col_pieces(w128, bound=512):
            # split [0, w128) at the 512-col psum bank boundary
            if w128 <= bound:
                return [(0, w128)]
            return [(0, bound), (bound, w128)]

        def scores(p):
            w = _width(p) * P
            ps = psb.tile([P, 1024], f32, tag="sc", name=f"sc_{p}", bufs=2)
            pt = ptp.tile([P, 1024], bf16, tag="pt", name=f"pt_{p}")
            pts[p] = pt
            kblk = KT[H:P, p * P:(p + 1) * P]
            for a, b in col_pieces(w):
                nc.tensor.matmul(
                    ps[:, a:b], kblk, QT[H:P, a:b],
                    start=True, stop=True, skip_group_check=True,
                )
            if p < 8:
                # own key: exp all, tri-mask the diagonal (last) block
                for a, b in col_pieces(w):
                    nc.scalar.activation(pt[:, a:b], ps[:, a:b], AF.Exp)
                nc.vector.tensor_tensor(
                    pt[:, w - P:w], pt[:, w - P:w], tri_sb, ALU.mult
                )
            else:
                # other key: last block fully causal or fully dead
                # (0/-30000 per-core exp bias)
                if w > P:
                    for a, b in col_pieces(w - P):
                        nc.scalar.activation(pt[:, a:b], ps[:, a:b], AF.Exp)
                nc.scalar.activation(
                    pt[:, w - P:w], ps[:, w - P:w], AF.Exp, bias=bg_sb(p)
                )

        def pv(p, stops=()):
            # start=True resets the ENTIRE 512-col psum bank, so each bank
            # gets exactly one start: unit 3 opens bank A with its full
            # [0:512] write, unit 7 opens bank B with [512:1024]; they are
            # emitted before any other writer of their bank.
            w = _width(p) * P
            pt = pts[p]
            if p == 3:
                pieces = [(0, 512, True)]
            elif p == 7:
                pieces = [(0, 512, False), (512, 1024, True)]
            else:
                pieces = [(a, b, False) for a, b in col_pieces(w)]
            for a, b, st in pieces:
                nc.tensor.matmul(
                    outT[:H + 1, a:b], Vp[:, p, :], pt[:, a:b],
                    start=st, stop=(a in stops),
                    skip_group_check=True,
                )

        def drain(a, b):
            nc.vector.tensor_copy(o_sb[:, a:b], outT[:H + 1, a:b])
            nc.sync.dma_start(out[:, a:b], o_sb[:, a:b])

        # --- emission order = per-engine FIFO order ---
        # 15 back-to-back N=512 warmups run dense from ~8.3us THROUGH the
        # weights-DMA landing (~12.7us) so the HAM utilization window never
        # dips: the full-clock grant opens just before projections start
        # and, with sustained utilization, stays open through attention.
        for i in range(15):
            wps = psb.tile([P, 512], f32, tag="pj", name=f"warm_{i}")
            nc.tensor.matmul(
                wps[:], wtile[:, 0:P], wtile[:],
                start=True, stop=True, skip_group_check=True,
            )

        # transposes and ready pvs are placed to fill the PE bubble while
        # each chunk's PSUM->SBUF copies (ACT/DVE) land
        vk_chunk(0)
        q_chunk(0)
        vtranspose(vts[0], 0)
        scores(3)
        scores(0)
        pv(3)
        scores(1)
        pv(0)
        scores(2)
        pv(1)
        pv(2)
        vk_chunk(1)
        q_chunk(1)
        vtranspose(vts[1], 1)
        scores(7)
        scores(4)
        pv(7)
        scores(5)
        pv(4)
        scores(6)
        pv(5)
        pv(6)
        vk_chunk(2, halves=True)
        vtranspose(vts[2], 2)
        scores(8)
        scores(9)
        pv(8)
        scores(10)
        pv(9)
        drain(768, 1024)
        scores(11)
        pv(10)
        vk_chunk(3, halves=True)
        pv(11, stops=(512,))
        vtranspose(vts[3], 3)
        scores(12)
        scores(13)
        pv(12)
        scores(14)
        pv(13)
        drain(256, 768)
        scores(15)
        pv(14)
        pv(15, stops=(0,))
        drain(0, 256)

    nc.finalize()
    return nc


_CACHED = None


def _get_program(zb):
    global _CACHED
    if _CACHED is None or _CACHED[0] != zb:
        _CACHED = (zb, _build_program(zb))
    return _CACHED[1]


def _host_inputs(embeddings, Wq, bq, Wk, bk, Wv, bv):
    import ml_dtypes

    bf16 = ml_dtypes.bfloat16
    tri = np.zeros((P, P), np.float32)
    for k in range(P):
        tri[k, k:] = 1.0
    ident = np.zeros((P, H), np.float32)
    ident[:H] = np.eye(H, dtype=np.float32)
    mi = np.ascontiguousarray(
        np.concatenate([tri, ident], axis=1)
    ).astype(bf16)

    def wlay(w):
        return np.asarray(w, np.float32).reshape(KO, P, H).transpose(1, 0, 2)

    wq8l = wlay(Wq) / 8.0
    wkl = wlay(Wk)
    wvl = wlay(Wv)
    wts = np.concatenate([wvl, wkl, wq8l], axis=2).reshape(P, 1536)
    bqf = np.asarray(bq, np.float32) / 8.0
    bkf = np.asarray(bk, np.float32)
    bvf = np.asarray(bv, np.float32)
    z64 = np.zeros(H, np.float32)
    bq8P = np.concatenate([z64, bqf])
    bkP = np.concatenate([z64, bkf])
    bvP = np.concatenate([bvf, z64])

    in_maps = []
    perms = []
    for c in range(8):
        b, h = c // 2, c % 2
        order = _order(h)
        own = set(OWN0 if h == 0 else OWN1)
        rows = np.concatenate(
            [np.arange(t * P, (t + 1) * P) for t in order]
        )
        perms.append(rows)
        ep = embeddings[b][rows]                      # [S, D] f32, permuted
        etl = np.ascontiguousarray(
            ep.T.reshape(KO, P, 4, 512).transpose(2, 1, 0, 3)
        ).astype(bf16).reshape(4, P, KO * 512)        # [cc, p, ko*512]
        # bg[p]: 0 if the last block of unit p is fully causal, NEG if dead
        bgs = []
        for p in range(8, 16):
            key = order[p]
            s = sum(1 for t in own if t >= key)
            bgs.append(
                np.full(P, 0.0 if s == _width(p) else NEG, np.float32)
            )
        biasg = np.ascontiguousarray(
            np.stack([bq8P, bkP, bvP] + bgs, axis=1)
        )
        wc0l = np.ascontiguousarray(
            np.concatenate([wts, etl[0]], axis=1)
        ).astype(bf16)
        in_maps.append({
            "et": etl, "wc0": wc0l, "biasg": biasg, "mi": mi,
        })
    return in_maps, perms


def _run(embeddings, Wq, bq, Wk, bk, Wv, bv, trace=False):
    from concourse.bass_utils import run_bass_kernel_spmd

    zb = (
        not np.any(np.asarray(bq)) and not np.any(np.asarray(bk))
        and not np.any(np.asarray(bv))
    )
    nc = _get_program(zb)
    in_maps, perms = _host_inputs(embeddings, Wq, bq, Wk, bk, Wv, bv)
    res = run_bass_kernel_spmd(
        nc, in_maps, core_ids=list(range(8)), trace=trace,
        trace_cores=list(range(8)) if trace else None,
    )
    full = np.empty((B, S, H), np.float32)
    for c in range(8):
        b = c // 2
        o = res.results[c]["out"]                     # [65, 1024] f32
        full[b, perms[c][:1024]] = (o[:H] / o[H:H + 1]).T
    return full, res


def kernel(embeddings, Wq, bq, Wk, bk, Wv, bv):
    full, _ = _run(
        np.asarray(embeddings, np.float32), Wq, bq, Wk, bk, Wv, bv, trace=False
    )
    return full


# revision 36
# speedup vs baseline: 1.1896x; 1.1896x over previous
"""Causal single-head attention on 8 trn2 NeuronCores - split-72 geometry.

B=4, S=2048, D_MODEL=1024, D_HEAD=64, fp32 in/out.

Sharding: 2 cores per batch with an interleaved query-tile split
(h=0 owns tiles {0,2,4,6,9,11,13,15}, h=1 the complement; 68 causal
128x128 blocks each). The host feeds each core E^T [dm, s] bf16 with
columns ordered [own tiles DESCENDING | other tiles ascending]. With
own-descending query columns, the queries needing key tile at position
p form a PREFIX of the 1024 QT columns, so each score unit computes a
prefix range:
  position p 0..7  (own keys):   width (p+1)*128, diag tri at last block
  position p 8..15 (other keys): width (16-p)*128, last block either
    fully causal or fully dead - killed by a per-core 0/-30000 exp bias
Total 72 blocks/core vs 84 for the contiguous-half split (68 = ideal).

Per-core pipeline (identical SPMD program, all matmuls bf16):
  Warmup N=512 matmuls open the PE HAM clock gate while the first input
  DMA is in flight; inputs stream over BOTH hardware DGE rings (Sync +
  Act). Projections per 512-col chunk of E^T: one [Wv|Wk]-packed pass
  (V^T on PSUM rows 0:64, K^T on 64:128) plus, for the core's own 2
  chunks, a Wq/8 pass targeting PSUM rows 64:128. Q^T/K^T live on SBUF
  partitions 64:128 (shared-base-partition rule); V tiles are
  PE-transposed into Vp [128k, 16, 65] with a ones column (softmax
  denominator). Zero biases -> all projection PSUM->SBUF copies are
  bias-free.
  PV accumulates out^T [65, 1024] in one 2-bank PSUM tile; start=True
  resets a whole 512-col psum bank, so the widest unit of each bank
  (3, 7) is emitted first and opens its bank with one full-bank start,
  everything else accumulates. Output drains in 3 pieces as column
  regions complete; the host does the final divide+transpose+scatter.
"""

import sys

if "/opt/trn_rl_repo" not in sys.path:
    sys.path.insert(0, "/opt/trn_rl_repo")

import numpy as np

B, S, D, H = 4, 2048, 1024, 64
P = 128
KO = D // P          # 8 dmodel chunks
NT = S // P          # 16 seq tiles
NEG = -30000.0
OWN0 = [0, 2, 4, 6, 9, 11, 13, 15]   # h=0 query tiles
OWN1 = [t for t in range(16) if t not in OWN0]


def _order(h):
    own = OWN0 if h == 0 else OWN1
    other = OWN1 if h == 0 else OWN0
    return sorted(own, reverse=True) + sorted(other)


def _width(p):
    return p + 1 if p < 8 else 16 - p


def _build_program(zb):
    import concourse.bacc as bacc
    import concourse.mybir as mybir
    import concourse.tile as tile

    f32 = mybir.dt.float32
    bf16 = mybir.dt.bfloat16
    AF = mybir.ActivationFunctionType
    ALU = mybir.AluOpType

    nc = bacc.Bacc()
    et = nc.declare_dram_parameter("et", [4, P, KO * 512], bf16, isOutput=False)
    # per partition cols 0:1536 = [Wv|Wk|Wq/8] x 8 ko, cols 1536:5632 = chunk0
    wc0 = nc.declare_dram_parameter("wc0", [P, 1536 + 4096], bf16, isOutput=False)
    # cols: bq/8 | bk | bv | bg[8..15] (0 or NEG per core)
    biasg = nc.declare_dram_parameter("biasg", [P, 11], f32, isOutput=False)
    # cols 0:128 = shared tri diag mask, cols 128:192 = identity (rows 0:64)
    mi = nc.declare_dram_parameter("mi", [P, P + H], bf16, isOutput=False)
    out = nc.declare_dram_parameter("out", [H + 1, 1024], f32, isOutput=True)

    from contextlib import ExitStack

    with tile.TileContext(nc) as tc, ExitStack() as ctx:
        cpool = ctx.enter_context(tc.tile_pool(name="const", bufs=1))
        vtp = ctx.enter_context(tc.tile_pool(name="vt", bufs=2))
        ptp = ctx.enter_context(tc.tile_pool(name="pt", bufs=10))
        psb = ctx.enter_context(tc.tile_pool(name="psb", bufs=2, space="PSUM"))

        # --- input DMAs: ONE hardware ring, strict need-order (both DGE
        # rings share HBM bandwidth; splitting starves the critical pieces)
        # NOTE each dma_start costs ~0.7us of serialized issue time on the
        # Sync queue - pieces must be >=~400KB (>=1.1us of stream) or the
        # DMA engine starves waiting for issues. Keep the count near 10.
        wc_sb = cpool.tile([P, 1536 + 4096], bf16, tag="wc0")
        nc.sync.dma_start(wc_sb[:, 0:1536], wc0[:, 0:1536])
        # chunk0 in four pieces so the first projection ko's unblock as
        # soon as their slice lands (dma deps are per-dma_start)
        for a in range(1536, 5632, 1024):
            b = min(a + 1024, 5632)
            nc.sync.dma_start(wc_sb[:, a:b], wc0[:, a:b])
        mi_sb = cpool.tile([P, P + H], bf16, tag="mi")
        nc.sync.dma_start(mi_sb[:], mi[:])
        bias_sb = cpool.tile([P, 11], f32, tag="biasg")
        nc.sync.dma_start(bias_sb[:], biasg[:])
        ET = cpool.tile([P, 4, KO, 512], bf16, tag="ET")
        # ET1 in ko-halves (contiguous 4KB/partition descriptors): chunk
        # 1's first matmuls start ~1.4us before the full chunk lands -
        # kills the straggler utilization dip that fragments the HAM
        # full-clock grant on DMA-jittered cores
        nc.sync.dma_start(ET[:, 1, 0:4, :], et[1, :, 0:2048])
        nc.sync.dma_start(ET[:, 1, 4:8, :], et[1, :, 2048:4096])
        nc.sync.dma_start(ET[:, 2, :, :], et[2, :, :])
        nc.sync.dma_start(ET[:, 3, :, :], et[3, :, :])

        def w_ap(ko, a, b):
            return wc_sb[:, ko * 192 + a:ko * 192 + b]

        def et_ap(cc, ko):
            if cc == 0:
                return wc_sb[:, 1536 + ko * 512:1536 + (ko + 1) * 512]
            return ET[:, cc, ko, :]

        bq_sb = bias_sb[:, 0:1]
        bk_sb = bias_sb[:, 1:2]
        bv_sb = bias_sb[:H, 2:3]

        def bg_sb(p):
            return bias_sb[:, 3 + (p - 8):4 + (p - 8)]

        tri_sb = mi_sb[:, 0:P]
        id_sb = mi_sb[:H, P:P + H]

        QT = cpool.tile([P, 1024], bf16, tag="QT")
        KT = cpool.tile([P, S], bf16, tag="KT")
        Vp = cpool.tile([P, NT, H + 1], bf16, tag="Vp")
        o_sb = cpool.tile([H + 1, 1024], f32, tag="osb")
        wtile = cpool.tile([P, 512], bf16, tag="warm")
        nc.vector.memset(wtile[:], 0.0)
        nc.vector.memset(Vp[:, :, H:H + 1], 1.0)

        def vtranspose(vt, cc):
            for t in range(4):
                kt = cc * 4 + t
                pvt = psb.tile([P, H], bf16, tag="pj", name=f"pvt_{kt}")
                nc.tensor.transpose(
                    pvt[:], vt[:, t * P:(t + 1) * P], id_sb[:]
                )
                nc.vector.tensor_copy(Vp[:, kt, :H], pvt[:])

        vts = [None] * 4

        def pcopy(dst, src_ap, bias, eng):
            if zb:
                if eng == "act":
                    nc.scalar.activation(dst, src_ap, AF.Copy)
                else:
                    nc.vector.tensor_copy(dst, src_ap)
            else:
                nc.vector.tensor_scalar_add(dst, src_ap, bias)

        def vk_chunk(cc, halves=False):
            # halves=True: two independent 256-col accumulation groups (in
            # SEPARATE psum tiles - start=True resets a whole bank) so the
            # first half's PSUM->SBUF copies overlap the second half's
            # matmuls - removes the chunk-transition PE bubble
            # KT/vt copies on DVE so they run concurrently with q_chunk's
            # ACT copy - the first scores then wait max(DVE, ACT) not sum
            eng = "dve"
            vt = vtp.tile([H, 512], bf16, tag="vt", name=f"vt_{cc}")
            grps = [(0, 256), (256, 512)] if halves else [(0, 512)]
            for a, b in grps:
                ps = psb.tile([P, b - a], f32, tag="pj",
                              name=f"vk_ps_{cc}_{a}")
                for ko in range(KO):
                    nc.tensor.matmul(
                        ps[:], w_ap(ko, 0, 128), et_ap(cc, ko)[:, a:b],
                        start=(ko == 0), stop=(ko == KO - 1),
                        skip_group_check=True,
                    )
                # KT copy in 128-col pieces: the next chunk's first score
                # unit needs only the first 128 cols, so it unblocks early
                for c0 in range(a, b, P):
                    pcopy(
                        KT[H:P, cc * 512 + c0:cc * 512 + c0 + P],
                        ps[H:P, c0 - a:c0 - a + P], bk_sb[H:P], eng,
                    )
                pcopy(vt[:, a:b], ps[:H, :], bv_sb[:], eng)
            vts[cc] = vt

        def q_chunk(cc):
            ps = psb.tile([P, 512], f32, tag="pj", name=f"q_ps_{cc}")
            for ko in range(KO):
                nc.tensor.matmul(
                    ps[H:P, :], w_ap(ko, 128, 192), et_ap(cc, ko),
                    start=(ko == 0), stop=(ko == KO - 1),
                )
            pcopy(
                QT[H:P, cc * 512:(cc + 1) * 512], ps[H:P, :], bq_sb[H:P],
                "act" if cc == 0 else "dve",
            )

        # --- attention: 16 prefix-range units over one 2-bank out^T psum
        outT = psb.tile([P, 1024], f32, tag="os", bufs=1)
        pts = [None] * NT

        def col_pieces(w128, bound=512):
            # split [0, w128) at the 512-col psum bank boundary
            if w128 <= bound:
                return [(0, w128)]
            return [(0, bound), (bound, w128)]

        def scores(p):
            w = _width(p) * P
            ps = psb.tile([P, 1024], f32, tag="sc", name=f"sc_{p}", bufs=2)
            pt = ptp.tile([P, 1024], bf16, tag="pt", name=f"pt_{p}")
            pts[p] = pt
            kblk = KT[H:P, p * P:(p + 1) * P]
            for a, b in col_pieces(w):
                nc.tensor.matmul(
                    ps[:, a:b], kblk, QT[H:P, a:b],
                    start=True, stop=True, skip_group_check=True,
                )
            if p < 8:
                # own key: exp all, tri-mask the diagonal (last) block
                for a, b in col_pieces(w):
                    nc.scalar.activation(pt[:, a:b], ps[:, a:b], AF.Exp)
                nc.vector.tensor_tensor(
                    pt[:, w - P:w], pt[:, w - P:w], tri_sb, ALU.mult
                )
            else:
                # other key: last block fully causal or fully dead
                # (0/-30000 per-core exp bias)
                if w > P:
                    for a, b in col_pieces(w - P):
                        nc.scalar.activation(pt[:, a:b], ps[:, a:b], AF.Exp)
                nc.scalar.activation(
                    pt[:, w - P:w], ps[:, w - P:w], AF.Exp, bias=bg_sb(p)
                )

        def pv(p, stops=()):
            # start=True resets the ENTIRE 512-col psum bank, so each bank
            # gets exactly one start: unit 3 opens bank A with its full
            # [0:512] write, unit 7 opens bank B with [512:1024]; they are
            # emitted before any other writer of their bank.
            w = _width(p) * P
            pt = pts[p]
            if p == 3:
                pieces = [(0, 512, True)]
            elif p == 7:
                pieces = [(0, 512, False), (512, 1024, True)]
            else:
                pieces = [(a, b, False) for a, b in col_pieces(w)]
            for a, b, st in pieces:
                nc.tensor.matmul(
                    outT[:H + 1, a:b], Vp[:, p, :], pt[:, a:b],
                    start=st, stop=(a in stops),
                    skip_group_check=True,
                )

        def drain(a, b):
            nc.vector.tensor_copy(o_sb[:, a:b], outT[:H + 1, a:b])
            nc.sync.dma_start(out[:, a:b], o_sb[:, a:b])

        # --- emission order = per-engine FIFO order ---
        # 15 back-to-back N=512 warmups run dense from ~8.3us THROUGH the
        # weights-DMA landing (~12.7us) so the HAM utilization window never
        # dips: the full-clock grant opens just before projections start
        # and, with sustained utilization, stays open through attention.
        for i in range(15):
            wps = psb.tile([P, 512], f32, tag="pj", name=f"warm_{i}")
            nc.tensor.matmul(
                wps[:], wtile[:, 0:P], wtile[:],
                start=True, stop=True, skip_group_check=True,
            )

        # transposes and ready pvs are placed to fill the PE bubble while
        # each chunk's PSUM->SBUF copies (ACT/DVE) land
        vk_chunk(0)
        q_chunk(0)
        vtranspose(vts[0], 0)
        scores(3)
        scores(0)
        pv(3)
        scores(1)
        pv(0)
        scores(2)
        pv(1)
        pv(2)
        vk_chunk(1)
        q_chunk(1)
        vtranspose(vts[1], 1)
        scores(7)
        scores(4)
        pv(7)
        scores(5)
        pv(4)
        scores(6)
        pv(5)
        pv(6)
        vk_chunk(2, halves=True)
        vtranspose(vts[2], 2)
        scores(8)
        scores(9)
        pv(8)
        scores(10)
        pv(9)
        drain(768, 1024)
        scores(11)
        pv(10)
        vk_chunk(3, halves=True)
        pv(11, stops=(512,))
        vtranspose(vts[3], 3)
        scores(12)
        scores(13)
        pv(12)
        scores(14)
        pv(13)
        drain(256, 768)
        scores(15)
        pv(14)
        pv(15, stops=(0,))
        drain(0, 256)

    nc.finalize()
    return nc


_CACHED = None


def _get_program(zb):
    global _CACHED
    if _CACHED is None or _CACHED[0] != zb:
        _CACHED = (zb, _build_program(zb))
    return _CACHED[1]


def _host_inputs(embeddings, Wq, bq, Wk, bk, Wv, bv):
    import ml_dtypes

    bf16 = ml_dtypes.bfloat16
    tri = np.zeros((P, P), np.float32)
    for k in range(P):
        tri[k, k:] = 1.0
    ident = np.zeros((P, H), np.float32)
    ident[:H] = np.eye(H, dtype=np.float32)
    mi = np.ascontiguousarray(
        np.concatenate([tri, ident], axis=1)
    ).astype(bf16)

    def wlay(w):
        return np.asarray(w, np.float32).reshape(KO, P, H).transpose(1, 0, 2)

    wq8l = wlay(Wq) / 8.0
    wkl = wlay(Wk)
    wvl = wlay(Wv)
    wts = np.concatenate([wvl, wkl, wq8l], axis=2).reshape(P, 1536)
    bqf = np.asarray(bq, np.float32) / 8.0
    bkf = np.asarray(bk, np.float32)
    bvf = np.asarray(bv, np.float32)
    z64 = np.zeros(H, np.float32)
    bq8P = np.concatenate([z64, bqf])
    bkP = np.concatenate([z64, bkf])
    bvP = np.concatenate([bvf, z64])

    in_maps = []
    perms = []
    for c in range(8):
        b, h = c // 2, c % 2
        order = _order(h)
        own = set(OWN0 if h == 0 else OWN1)
        rows = np.concatenate(
            [np.arange(t * P, (t + 1) * P) for t in order]
        )
        perms.append(rows)
        ep = embeddings[b][rows]                      # [S, D] f32, permuted
        etl = np.ascontiguousarray(
            ep.T.reshape(KO, P, 4, 512).transpose(2, 1, 0, 3)
        ).astype(bf16).reshape(4, P, KO * 512)        # [cc, p, ko*512]
        # bg[p]: 0 if the last block of unit p is fully causal, NEG if dead
        bgs = []
        for p in range(8, 16):
            key = order[p]
            s = sum(1 for t in own if t >= key)
            bgs.append(
                np.full(P, 0.0 if s == _width(p) else NEG, np.float32)
            )
        biasg = np.ascontiguousarray(
            np.stack([bq8P, bkP, bvP] + bgs, axis=1)
        )
        wc0l = np.ascontiguousarray(
            np.concatenate([wts, etl[0]], axis=1)
        ).astype(bf16)
        in_maps.append({
            "et": etl, "wc0": wc0l, "biasg": biasg, "mi": mi,
        })
    return in_maps, perms


def _run(embeddings, Wq, bq, Wk, bk, Wv, bv, trace=False):
    from concourse.bass_utils import run_bass_kernel_spmd

    zb = (
        not np.any(np.asarray(bq)) and not np.any(np.asarray(bk))
        and not np.any(np.asarray(bv))
    )
    nc = _get_program(zb)
    in_maps, perms = _host_inputs(embeddings, Wq, bq, Wk, bk, Wv, bv)
    res = run_bass_kernel_spmd(
        nc, in_maps, core_ids=list(range(8)), trace=trace,
        trace_cores=list(range(8)) if trace else None,
    )
    full = np.empty((B, S, H), np.float32)
    for c in range(8):
        b = c // 2
        o = res.results[c]["out"]                     # [65, 1024] f32
        full[b, perms[c][:1024]] = (o[:H] / o[H:H + 1]).T
    return full, res


def kernel(embeddings, Wq, bq, Wk, bk, Wv, bv):
    full, _ = _run(
        np.asarray(embeddings, np.float32), Wq, bq, Wk, bk, Wv, bv, trace=False
    )
    return full


# revision 41
# speedup vs baseline: 1.2093x; 1.0165x over previous
"""Causal single-head attention on 8 trn2 NeuronCores - split-72 geometry.

B=4, S=2048, D_MODEL=1024, D_HEAD=64, fp32 in/out.

Sharding: 2 cores per batch with an interleaved query-tile split
(h=0 owns tiles {0,2,4,6,9,11,13,15}, h=1 the complement; 68 causal
128x128 blocks each). The host feeds each core E^T [dm, s] bf16 with
columns ordered [own tiles DESCENDING | other tiles ascending]. With
own-descending query columns, the queries needing key tile at position
p form a PREFIX of the 1024 QT columns, so each score unit computes a
prefix range:
  position p 0..7  (own keys):   width (p+1)*128, diag tri at last block
  position p 8..15 (other keys): width (16-p)*128, last block either
    fully causal or fully dead - killed by a per-core 0/-30000 exp bias
Total 72 blocks/core vs 84 for the contiguous-half split (68 = ideal).

Per-core pipeline (identical SPMD program, all matmuls bf16):
  Warmup N=512 matmuls open the PE HAM clock gate while the first input
  DMA is in flight; inputs stream over ONE hardware DGE ring in strict
  need-order. Projections per 512-col chunk of E^T: one [Wv|Wk]-packed pass
  (V^T on PSUM rows 0:64, K^T on 64:128) plus, for the core's own 2
  chunks, a Wq/8 pass targeting PSUM rows 64:128. Q^T/K^T live on SBUF
  partitions 64:128 (shared-base-partition rule); V tiles are
  PE-transposed into Vp [128k, 16, 65] with a ones column (softmax
  denominator). Zero biases -> all projection PSUM->SBUF copies are
  bias-free.
  PV accumulates out^T [65, 1024] in one 2-bank PSUM tile; start=True
  resets a whole 512-col psum bank, so the widest unit of each bank
  (3, 7) is emitted first and opens its bank with one full-bank start,
  everything else accumulates. Output drains in 3 pieces as column
  regions complete; the host does the final divide+transpose+scatter.
"""

import sys

if "/opt/trn_rl_repo" not in sys.path:
    sys.path.insert(0, "/opt/trn_rl_repo")

import numpy as np

B, S, D, H = 4, 2048, 1024, 64
P = 128
KO = D // P          # 8 dmodel chunks
NT = S // P          # 16 seq tiles
NEG = -30000.0
OWN0 = [0, 2, 4, 6, 9, 11, 13, 15]   # h=0 query tiles
OWN1 = [t for t in range(16) if t not in OWN0]


def _order(h):
    own = OWN0 if h == 0 else OWN1
    other = OWN1 if h == 0 else OWN0
    return sorted(own, reverse=True) + sorted(other)


def _width(p):
    return p + 1 if p < 8 else 16 - p


def _build_program(zb):
    import concourse.bacc as bacc
    import concourse.mybir as mybir
    import concourse.tile as tile

    f32 = mybir.dt.float32
    bf16 = mybir.dt.bfloat16
    AF = mybir.ActivationFunctionType
    ALU = mybir.AluOpType

    nc = bacc.Bacc()
    et = nc.declare_dram_parameter("et", [4, P, KO * 512], bf16, isOutput=False)
    # per partition cols 0:1536 = [Wv|Wk|Wq/8] x 8 ko, cols 1536:5632 = chunk0
    wc0 = nc.declare_dram_parameter("wc0", [P, 1536 + 4096], bf16, isOutput=False)
    # cols: bq/8 | bk | bv | bg[8..15] (0 or NEG per core)
    biasg = nc.declare_dram_parameter("biasg", [P, 11], f32, isOutput=False)
    # cols 0:128 = shared tri diag mask, cols 128:192 = identity (rows 0:64)
    mi = nc.declare_dram_parameter("mi", [P, P + H], bf16, isOutput=False)
    out = nc.declare_dram_parameter("out", [H + 1, 1024], f32, isOutput=True)

    from contextlib import ExitStack

    with tile.TileContext(nc) as tc, ExitStack() as ctx:
        cpool = ctx.enter_context(tc.tile_pool(name="const", bufs=1))
        vtp = ctx.enter_context(tc.tile_pool(name="vt", bufs=2))
        ptp = ctx.enter_context(tc.tile_pool(name="pt", bufs=10))
        psb = ctx.enter_context(tc.tile_pool(name="psb", bufs=2, space="PSUM"))

        # --- input DMAs: ONE hardware ring, strict need-order (both DGE
        # rings share HBM bandwidth; splitting starves the critical pieces)
        wc_sb = cpool.tile([P, 1536 + 4096], bf16, tag="wc0")
        nc.sync.dma_start(wc_sb[:, 0:1536], wc0[:, 0:1536])
        # chunk0 in four pieces so the first projection ko's unblock as
        # soon as their slice lands (dma deps are per-dma_start)
        for a in range(1536, 5632, 1024):
            b = min(a + 1024, 5632)
            nc.sync.dma_start(wc_sb[:, a:b], wc0[:, a:b])
        mi_sb = cpool.tile([P, P + H], bf16, tag="mi")
        nc.sync.dma_start(mi_sb[:], mi[:])
        bias_sb = cpool.tile([P, 11], f32, tag="biasg")
        nc.sync.dma_start(bias_sb[:], biasg[:])
        ET = cpool.tile([P, 4, KO, 512], bf16, tag="ET")
        # ET1 in ko-halves (contiguous 4KB/partition descriptors): chunk
        # 1's first matmuls start ~1.4us before the full chunk lands,
        # closing the utilization dip that fragments the HAM full-clock
        # grant on DMA-jittered cores. (Each dma_start costs ~0.7us of
        # serialized issue time on the Sync queue - keep pieces >=400KB.)
        nc.sync.dma_start(ET[:, 1, 0:4, :], et[1, :, 0:2048])
        nc.sync.dma_start(ET[:, 1, 4:8, :], et[1, :, 2048:4096])
        nc.sync.dma_start(ET[:, 2, :, :], et[2, :, :])
        nc.sync.dma_start(ET[:, 3, :, :], et[3, :, :])

        def w_ap(ko, a, b):
            return wc_sb[:, ko * 192 + a:ko * 192 + b]

        def et_ap(cc, ko):
            if cc == 0:
                return wc_sb[:, 1536 + ko * 512:1536 + (ko + 1) * 512]
            return ET[:, cc, ko, :]

        bq_sb = bias_sb[:, 0:1]
        bk_sb = bias_sb[:, 1:2]
        bv_sb = bias_sb[:H, 2:3]

        def bg_sb(p):
            return bias_sb[:, 3 + (p - 8):4 + (p - 8)]

        tri_sb = mi_sb[:, 0:P]
        id_sb = mi_sb[:H, P:P + H]

        QT = cpool.tile([P, 1024], bf16, tag="QT")
        KT = cpool.tile([P, S], bf16, tag="KT")
        Vp = cpool.tile([P, NT, H + 1], bf16, tag="Vp")
        o_sb = cpool.tile([H + 1, 1024], f32, tag="osb")
        wtile = cpool.tile([P, 512], bf16, tag="warm")
        nc.vector.memset(wtile[:], 0.0)
        nc.vector.memset(Vp[:, :, H:H + 1], 1.0)

        def vtranspose(vt, cc):
            for t in range(4):
                kt = cc * 4 + t
                pvt = psb.tile([P, H], bf16, tag="pj", name=f"pvt_{kt}")
                nc.tensor.transpose(
                    pvt[:], vt[:, t * P:(t + 1) * P], id_sb[:]
                )
                nc.vector.tensor_copy(Vp[:, kt, :H], pvt[:])

        vts = [None] * 4

        def pcopy(dst, src_ap, bias, eng):
            if zb:
                if eng == "act":
                    nc.scalar.activation(dst, src_ap, AF.Copy)
                else:
                    nc.vector.tensor_copy(dst, src_ap)
            else:
                nc.vector.tensor_scalar_add(dst, src_ap, bias)

        def vk_chunk(cc, halves=False):
            # halves=True: two independent 256-col accumulation groups (in
            # SEPARATE psum tiles - start=True resets a whole bank) so the
            # first half's PSUM->SBUF copies overlap the second half's
            # matmuls - removes the chunk-transition PE bubble
            # KT/vt copies on DVE so they run concurrently with q_chunk's
            # ACT copy - the first scores then wait max(DVE, ACT) not sum
            eng = "dve"
            vt = vtp.tile([H, 512], bf16, tag="vt", name=f"vt_{cc}")
            grps = [(0, 256), (256, 512)] if halves else [(0, 512)]
            for a, b in grps:
                ps = psb.tile([P, b - a], f32, tag="pj",
                              name=f"vk_ps_{cc}_{a}")
                for ko in range(KO):
                    nc.tensor.matmul(
                        ps[:], w_ap(ko, 0, 128), et_ap(cc, ko)[:, a:b],
                        start=(ko == 0), stop=(ko == KO - 1),
                        skip_group_check=True,
                    )
                # KT copy in 128-col pieces: the next chunk's first score
                # unit needs only the first 128 cols, so it unblocks early
                for c0 in range(a, b, P):
                    pcopy(
                        KT[H:P, cc * 512 + c0:cc * 512 + c0 + P],
                        ps[H:P, c0 - a:c0 - a + P], bk_sb[H:P], eng,
                    )
                pcopy(vt[:, a:b], ps[:H, :], bv_sb[:], eng)
            vts[cc] = vt

        def q_chunk(cc):
            ps = psb.tile([P, 512], f32, tag="pj", name=f"q_ps_{cc}")
            for ko in range(KO):
                nc.tensor.matmul(
                    ps[H:P, :], w_ap(ko, 128, 192), et_ap(cc, ko),
                    start=(ko == 0), stop=(ko == KO - 1),
                )
            pcopy(
                QT[H:P, cc * 512:(cc + 1) * 512], ps[H:P, :], bq_sb[H:P],
                "act" if cc == 0 else "dve",
            )

        # --- attention: 16 prefix-range units over one 2-bank out^T psum
        outT = psb.tile([P, 1024], f32, tag="os", bufs=1)
        pts = [None] * NT

        def col_pieces(w128, bound=512):
            # split [0, w128) at the 512-col psum bank boundary
            if w128 <= bound:
                return [(0, w128)]
            return [(0, bound), (bound, w128)]

        def scores(p):
            w = _width(p) * P
            ps = psb.tile([P, 1024], f32, tag="sc", name=f"sc_{p}", bufs=2)
            pt = ptp.tile([P, 1024], bf16, tag="pt", name=f"pt_{p}")
            pts[p] = pt
            kblk = KT[H:P, p * P:(p + 1) * P]
            for a, b in col_pieces(w):
                nc.tensor.matmul(
                    ps[:, a:b], kblk, QT[H:P, a:b],
                    start=True, stop=True, skip_group_check=True,
                )
            if p < 8:
                # own key: exp all, tri-mask the diagonal (last) block
                for a, b in col_pieces(w):
                    nc.scalar.activation(pt[:, a:b], ps[:, a:b], AF.Exp)
                nc.vector.tensor_tensor(
                    pt[:, w - P:w], pt[:, w - P:w], tri_sb, ALU.mult
                )
            else:
                # other key: last block fully causal or fully dead
                # (0/-30000 per-core exp bias)
                if w > P:
                    for a, b in col_pieces(w - P):
                        nc.scalar.activation(pt[:, a:b], ps[:, a:b], AF.Exp)
                nc.scalar.activation(
                    pt[:, w - P:w], ps[:, w - P:w], AF.Exp, bias=bg_sb(p)
                )

        def pv(p, stops=()):
            # start=True resets the ENTIRE 512-col psum bank, so each bank
            # gets exactly one start: unit 3 opens bank A with its full
            # [0:512] write, unit 7 opens bank B with [512:1024]; they are
            # emitted before any other writer of their bank.
            w = _width(p) * P
            pt = pts[p]
            if p == 3:
                pieces = [(0, 512, True)]
            elif p == 7:
                pieces = [(0, 512, False), (512, 1024, True)]
            else:
                pieces = [(a, b, False) for a, b in col_pieces(w)]
            for a, b, st in pieces:
                nc.tensor.matmul(
                    outT[:H + 1, a:b], Vp[:, p, :], pt[:, a:b],
                    start=st, stop=(a in stops),
                    skip_group_check=True,
                )

        def drain(a, b):
            nc.vector.tensor_copy(o_sb[:, a:b], outT[:H + 1, a:b])
            nc.sync.dma_start(out[:, a:b], o_sb[:, a:b])

        # --- emission order = per-engine FIFO order ---
        # 13 back-to-back N=512 warmups run dense from ~8.3us THROUGH the
        # weights-DMA landing (~12.7us) so the HAM utilization window never
        # dips: the full-clock grant opens just before projections start
        # and, with sustained utilization, stays open through attention.
        for i in range(13):
            wps = psb.tile([P, 512], f32, tag="pj", name=f"warm_{i}")
            nc.tensor.matmul(
                wps[:], wtile[:, 0:P], wtile[:],
                start=True, stop=True, skip_group_check=True,
            )

        # transposes and ready pvs are placed to fill the PE bubble while
        # each chunk's PSUM->SBUF copies (ACT/DVE) land
        vk_chunk(0)
        q_chunk(0)
        vtranspose(vts[0], 0)
        scores(3)
        scores(0)
        pv(3)
        scores(1)
        pv(0)
        scores(2)
        pv(1)
        pv(2)
        vk_chunk(1)
        q_chunk(1)
        vtranspose(vts[1], 1)
        scores(7)
        scores(4)
        pv(7)
        scores(5)
        pv(4)
        scores(6)
        pv(5)
        pv(6)
        vk_chunk(2, halves=True)
        vtranspose(vts[2], 2)
        scores(8)
        scores(9)
        pv(8)
        scores(10)
        pv(9)
        drain(768, 1024)
        scores(11)
        pv(10)
        vk_chunk(3, halves=True)
        pv(11, stops=(512,))
        vtranspose(vts[3], 3)
        scores(12)
        scores(13)
        pv(12)
        scores(14)
        pv(13)
        drain(256, 768)
        scores(15)
        pv(14)
        pv(15, stops=(0,))
        drain(0, 256)

    nc.finalize()
    return nc


_CACHED = None


def _get_program(zb):
    global _CACHED
    if _CACHED is None or _CACHED[0] != zb:
        _CACHED = (zb, _build_program(zb))
    return _CACHED[1]


def _host_inputs(embeddings, Wq, bq, Wk, bk, Wv, bv):
    import ml_dtypes

    bf16 = ml_dtypes.bfloat16
    tri = np.zeros((P, P), np.float32)
    for k in range(P):
        tri[k, k:] = 1.0
    ident = np.zeros((P, H), np.float32)
    ident[:H] = np.eye(H, dtype=np.float32)
    mi = np.ascontiguousarray(
        np.concatenate([tri, ident], axis=1)
    ).astype(bf16)

    def wlay(w):
        return np.asarray(w, np.float32).reshape(KO, P, H).transpose(1, 0, 2)

    wq8l = wlay(Wq) / 8.0
    wkl = wlay(Wk)
    wvl = wlay(Wv)
    wts = np.concatenate([wvl, wkl, wq8l], axis=2).reshape(P, 1536)
    bqf = np.asarray(bq, np.float32) / 8.0
    bkf = np.asarray(bk, np.float32)
    bvf = np.asarray(bv, np.float32)
    z64 = np.zeros(H, np.float32)
    bq8P = np.concatenate([z64, bqf])
    bkP = np.concatenate([z64, bkf])
    bvP = np.concatenate([bvf, z64])

    in_maps = []
    perms = []
    for c in range(8):
        b, h = c // 2, c % 2
        order = _order(h)
        own = set(OWN0 if h == 0 else OWN1)
        rows = np.concatenate(
            [np.arange(t * P, (t + 1) * P) for t in order]
        )
        perms.append(rows)
        ep = embeddings[b][rows]                      # [S, D] f32, permuted
        etl = np.ascontiguousarray(
            ep.T.reshape(KO, P, 4, 512).transpose(2, 1, 0, 3)
        ).astype(bf16).reshape(4, P, KO * 512)        # [cc, p, ko*512]
        # bg[p]: 0 if the last block of unit p is fully causal, NEG if dead
        bgs = []
        for p in range(8, 16):
            key = order[p]
            s = sum(1 for t in own if t >= key)
            bgs.append(
                np.full(P, 0.0 if s == _width(p) else NEG, np.float32)
            )
        biasg = np.ascontiguousarray(
            np.stack([bq8P, bkP, bvP] + bgs, axis=1)
        )
        wc0l = np.ascontiguousarray(
            np.concatenate([wts, etl[0]], axis=1)
        ).astype(bf16)
        in_maps.append({
            "et": etl, "wc0": wc0l, "biasg": biasg, "mi": mi,
        })
    return in_maps, perms


def _run(embeddings, Wq, bq, Wk, bk, Wv, bv, trace=False):
    from concourse.bass_utils import run_bass_kernel_spmd

    zb = (
        not np.any(np.asarray(bq)) and not np.any(np.asarray(bk))
        and not np.any(np.asarray(bv))
    )
    nc = _get_program(zb)
    in_maps, perms = _host_inputs(embeddings, Wq, bq, Wk, bk, Wv, bv)
    res = run_bass_kernel_spmd(
        nc, in_maps, core_ids=list(range(8)), trace=trace,
        trace_cores=list(range(8)) if trace else None,
    )
    full = np.empty((B, S, H), np.float32)
    for c in range(8):
        b = c // 2
        o = res.results[c]["out"]                     # [65, 1024] f32
        full[b, perms[c][:1024]] = (o[:H] / o[H:H + 1]).T
    return full, res


def kernel(embeddings, Wq, bq, Wk, bk, Wv, bv):
    full, _ = _run(
        np.asarray(embeddings, np.float32), Wq, bq, Wk, bk, Wv, bv, trace=False
    )
    return full
